# revision 1
# baseline (speedup 1.0000x reference)
"""CQT layer kernel for Trainium2 (8 NeuronCores, SPMD).

The strided conv (hop 128 == PE contraction tile) is a chunked matmul:
  out[c, b, t] = sum_k  W[c, 128k:128k+128] . xT_b[:, t+k]
where xT_b is the zero-padded audio reshaped to [128, 672] (a free reshape,
because hop == 128).  Each core holds 128 of the 1056 output channels as the
stationary operand (full PE array) and streams the frame columns; the 32
leftover channels are split across cores by chunk range and summed on host.
Magnitude + power_to_db run on host, with an exact fp64 recompute of the few
near-silent bins where reduced-precision matmul error would be audible in dB.

Self-contained: only needs numpy + the concourse toolchain at /opt/trn_rl_repo.
"""
import os
import sys

sys.path.insert(0, "/opt/trn_rl_repo")
import numpy as np

# ---- problem constants (hardcoded from the CQT layer spec) ----
B = 2
AUDIO_LEN = 22016
N_BINS = 528
NCH = 2 * N_BINS          # 1056 conv channels (re, im)
HOP = 128
FRAMES = 173
AMIN = 1e-10
TOP_DB = 80.0

K = 128                   # PE contraction tile == HOP
NCHUNK = 499              # ceil(L / 128); holds for L in [63745, 63872]
LPAD = NCHUNK * K         # 63872
NT = 174                  # frames padded to even (fp32r needs even free dims)
NROW = NCHUNK + NT - 1    # 672 columns of xT per batch
N_CORES = 8
MAIN_CH = 128             # stationary channels per core
TAIL_CH = NCH - N_CORES * MAIN_CH   # 32
TPC = 63                  # tail chunks per core (8*63 = 504 >= 499)
TCOLS = TPC + NT - 1      # 236 xT columns each core needs for its tail window
GROUP = int(os.environ.get("CQT_GROUP", "32"))  # weight chunks per DMA group

DTYPE = os.environ.get("CQT_DTYPE", "float16")  # float16 | float32r
# device matmul relative error (vs conv rms); drives the host refinement
# threshold for near-silent bins.  abs_err ~= eps * rms(conv) because the
# per-product rounding errors accumulate like the products themselves.
_CONV_EPS = {"float16": 1e-3, "float32r": 5e-4, "bfloat16": 5e-3}
DB_ERR_TARGET = 0.02      # refine bins whose worst-case dB error exceeds this

_prog_cache = {}


def _np_cast(a):
    if DTYPE == "float16":
        return a.astype(np.float16)
    if DTYPE == "bfloat16":
        import ml_dtypes
        return a.astype(ml_dtypes.bfloat16)
    return a  # float32r: raw fp32 bits


def _build_program():
    from concourse import bacc, mybir
    from concourse.tile import TileContext

    dt = mybir.dt
    DT = getattr(dt, DTYPE)

    nc = bacc.Bacc(None, target_bir_lowering=False)
    xt_p = nc.declare_dram_parameter("xt", [K, B * NROW], DT, isOutput=False)
    xtl_p = nc.declare_dram_parameter("xtl", [K, B * TCOLS], DT, isOutput=False)
    wm_p = nc.declare_dram_parameter("wm", [K, NCHUNK * MAIN_CH], DT, isOutput=False)
    wt_p = nc.declare_dram_parameter("wt", [K, TPC * TAIL_CH], DT, isOutput=False)
    om_p = nc.declare_dram_parameter("om", [MAIN_CH, B * NT], dt.float32, isOutput=True)
    ot_p = nc.declare_dram_parameter("ot", [MAIN_CH, B * NT], dt.float32, isOutput=True)

    # main weight groups: small first so the PE starts streaming early,
    # then 32-chunk (1 MB) steady-state DMAs
    groups = []
    k0 = 0
    ramp = [int(v) for v in os.environ.get("CQT_RAMP", "4,8,16").split(",") if v]
    for g in ramp:
        groups.append((k0, g))
        k0 += g
    while k0 < NCHUNK:
        cnt = min(GROUP, NCHUNK - k0)
        groups.append((k0, cnt))
        k0 += cnt
    XP2_AFTER = 3          # second half of x frames rides behind early groups
    TAIL_INPUT_AFTER = 6   # issue tail-input DMAs once main supply is ahead
    TAIL_MM_AFTER = 8      # run tail matmuls mid-stream; epilogue overlaps main
    N_WARM = int(os.environ.get("CQT_WARM", "10"))  # HAM warm-up matmuls

    with TileContext(nc) as tc:
        with (
            tc.tile_pool(name="stat", bufs=1) as stat,
            tc.tile_pool(name="wpool", bufs=4) as wpool,
            tc.tile_pool(name="opool", bufs=1) as opool,
            tc.tile_pool(name="ps", bufs=1, space="PSUM") as ps,
        ):
            # PE warm-up on a memset tile: no DMA dependency, runs during
            # the input-DMA window so HAM reaches 2.4 GHz before real work
            warm_sb = stat.tile([K, B * NT], DT)
            nc.gpsimd.memset(warm_sb[:], 0.0)
            ps_warm = ps.tile([K, B * NT], dt.float32)
            for _ in range(N_WARM):
                nc.tensor.matmul(ps_warm[:], warm_sb[:, :K], warm_sb[:],
                                 start=True, stop=True)

            # critical-path inputs: first half of the (t,b)-interleaved x
            # frames (enough for chunks 0..162), then ramped weight groups;
            # the rest of x and the tail inputs ride behind.
            xt_sb = stat.tile([K, B * NROW], DT)
            nc.sync.dma_start(xt_sb[:, :NROW], xt_p[:, :NROW])
            xtl_sb = stat.tile([K, B * TCOLS], DT)
            wt_sb = stat.tile([K, TPC * TAIL_CH], DT)
            wgs = []
            for gi, (g0, cnt) in enumerate(groups):
                wg = wpool.tile([K, GROUP * MAIN_CH], DT, tag="wg")
                nc.sync.dma_start(
                    wg[:, :cnt * MAIN_CH],
                    wm_p[:, g0 * MAIN_CH:(g0 + cnt) * MAIN_CH],
                )
                wgs.append(wg)
                if gi == XP2_AFTER:
                    nc.sync.dma_start(xt_sb[:, NROW:], xt_p[:, NROW:])
                if gi == TAIL_INPUT_AFTER:
                    nc.sync.dma_start(xtl_sb[:], xtl_p[:])
                    nc.sync.dma_start(wt_sb[:], wt_p[:])

            x3 = xt_sb[:].rearrange("p (t b) -> p t b", b=B)
            xl3 = xtl_sb[:].rearrange("p (t b) -> p t b", b=B)

            ps_main = ps.tile([MAIN_CH, B * NT], dt.float32)
            pm3 = ps_main[:].rearrange("p (t b) -> p t b", b=B)
            ps_tg = []
            for g in range(4):
                ptile = ps.tile([MAIN_CH, B * NT], dt.float32, tag=f"pt{g}", name=f"pt{g}")
                ps_tg.append(ptile)
            pt3g = [p[32 * g:32 * (g + 1), :].rearrange("p (t b) -> p t b", b=B)
                    for g, p in enumerate(ps_tg)]
            ot_sb = opool.tile([MAIN_CH, B * NT], dt.float32)

            def tail_block():
                # 32 channels x 63 chunks, 4-way column-tiled: four M=32
                # matmuls run concurrently in distinct PE column groups,
                # each accumulating in its own PSUM bank at partitions
                # [32g, 32g+32); host sums the four partials.
                for j in range(TPC):
                    g = j % 4
                    nc.tensor.matmul(
                        pt3g[g],
                        wt_sb[:, j * TAIL_CH:(j + 1) * TAIL_CH],
                        xl3[:, j:j + NT, :],
                        start=(j < 4),
                        stop=(j + 4 >= TPC),
                        tile_position=(0, 32 * g),
                    )
                for g in range(4):
                    nc.vector.tensor_copy(
                        ot_sb[32 * g:32 * (g + 1), :],
                        ps_tg[g][32 * g:32 * (g + 1), :],
                    )
                nc.sync.dma_start(ot_p[:], ot_sb[:])

            # main: 128 stationary channels x 499 chunks, weights streamed
            for gi, ((g0, cnt), wg) in enumerate(zip(groups, wgs)):
                for j in range(cnt):
                    k = g0 + j
                    nc.tensor.matmul(
                        pm3,
                        wg[:, j * MAIN_CH:(j + 1) * MAIN_CH],
                        x3[:, k:k + NT, :],
                        start=(k == 0),
                        stop=(k == NCHUNK - 1),
                    )
                if gi == TAIL_MM_AFTER:
                    tail_block()

            # drain main PSUM in halves so the copy and the out-DMA pipeline
            om_sb = opool.tile([MAIN_CH, B * NT], dt.float32)
            half = B * NT // 2
            for h in range(2):
                sl = slice(h * half, (h + 1) * half)
                nc.vector.tensor_copy(om_sb[:, sl], ps_main[:, sl])
                nc.sync.dma_start(om_p[:, sl], om_sb[:, sl])

    nc.finalize()
    return nc


LAST_RESULTS = None


def kernel(y, kern_r, kern_i):
    global LAST_RESULTS
    from concourse.bass_utils import run_bass_kernel_spmd

    y = np.asarray(y, dtype=np.float32)
    kern_r = np.asarray(kern_r, dtype=np.float32)
    kern_i = np.asarray(kern_i, dtype=np.float32)

    # ---- host prep: weights -> [l, (chunk, ch)] layouts ----
    L_in = kern_r.shape[1]                                 # 63864 from the layer
    pad = L_in // 2
    assert (NCHUNK - 1) * K < L_in <= LPAD, L_in
    W = np.concatenate([kern_r, kern_i], axis=0)          # [1056, L]
    Wp = np.zeros((NCH, LPAD), np.float32)
    Wp[:, :L_in] = W
    Wk = Wp.reshape(NCH, NCHUNK, K)                        # [c, k, l]

    # ---- host prep: audio -> xT [128, per-batch 672 cols] ----
    x_pad = np.zeros((B, NROW * K), np.float32)
    x_pad[:, pad:pad + AUDIO_LEN] = y
    xT = np.ascontiguousarray(x_pad.reshape(B, NROW, K).transpose(0, 2, 1))  # [B,128,672]
    # (t, b)-interleaved columns: col 2t+b = xT[b][:, t]
    xt_il = np.empty((K, B * NROW), np.float32)
    xt_il[:, 0::2] = xT[0]
    xt_il[:, 1::2] = xT[1]
    xt_host = _np_cast(xt_il)                                                # [128, 1344]

    in_maps = []
    for i in range(N_CORES):
        # main weights: channels [128i, 128i+128), all chunks -> [128l, 499*128]
        wm = np.ascontiguousarray(
            Wk[i * MAIN_CH:(i + 1) * MAIN_CH].transpose(2, 1, 0)
        ).reshape(K, NCHUNK * MAIN_CH)
        # tail weights: channels 1024.., chunks [63i, 63i+63) (zero-padded)
        wt = np.zeros((K, TPC, TAIL_CH), np.float32)
        k0, k1 = i * TPC, min((i + 1) * TPC, NCHUNK)
        if k1 > k0:
            wt[:, :k1 - k0, :] = Wk[N_CORES * MAIN_CH:, k0:k1, :].transpose(2, 1, 0)
        wt = wt.reshape(K, TPC * TAIL_CH)
        # tail x window: xT columns [63i, 63i+236), zero-padded past 672
        xtl = np.zeros((B, K, TCOLS), np.float32)
        hi = min(NROW, i * TPC + TCOLS)
        if hi > i * TPC:
            xtl[:, :, :hi - i * TPC] = xT[:, :, i * TPC:hi]
        xtl_host = np.empty((K, B * TCOLS), np.float32)
        xtl_host[:, 0::2] = xtl[0]
        xtl_host[:, 1::2] = xtl[1]
        in_maps.append({
            "xt": xt_host,
            "xtl": _np_cast(np.ascontiguousarray(xtl_host)),
            "wm": _np_cast(wm),
            "wt": _np_cast(np.ascontiguousarray(wt)),
        })

    if DTYPE not in _prog_cache:
        _prog_cache[DTYPE] = _build_program()
    nc = _prog_cache[DTYPE]

    LAST_RESULTS = run_bass_kernel_spmd(
        nc, in_maps, list(range(N_CORES)),
        trace=bool(os.environ.get("CQT_TRACE")),
    )
    results = LAST_RESULTS.results

    # ---- host post: assemble conv, magnitude, power_to_db ----
    conv = np.zeros((NCH, B, FRAMES), np.float32)
    tail = np.zeros((TAIL_CH, B, NT), np.float64)
    for i in range(N_CORES):
        om = results[i]["om"].reshape(MAIN_CH, NT, B).transpose(0, 2, 1)
        conv[i * MAIN_CH:(i + 1) * MAIN_CH] = om[:, :, :FRAMES]
        # [128, NT, B] -> 4 column-group partials at partitions [32g, 32g+32)
        tail += results[i]["ot"].reshape(4, TAIL_CH, NT, B).sum(axis=0).transpose(0, 2, 1)
    conv[N_CORES * MAIN_CH:] = tail[:, :, :FRAMES].astype(np.float32)

    re = conv[:N_BINS].astype(np.float64)                  # [528, B, 173]
    im = conv[N_BINS:].astype(np.float64)
    mag = np.sqrt(re * re + im * im)                       # [528, B, 173]

    # ---- host refinement: exact recompute of near-silent bins ----
    conv_rms = float(np.sqrt(np.mean(mag * mag)))
    err_abs = _CONV_EPS.get(DTYPE, 1e-3) * conv_rms
    thresh = 4.343 * err_abs / DB_ERR_TARGET
    fix = np.argwhere(mag < thresh)                        # rows: (bin, b, t)
    if len(fix):
        W64 = W.astype(np.float64)
        xp64 = x_pad.astype(np.float64)
        for b in range(B):
            sel = fix[fix[:, 1] == b]
            if not len(sel):
                continue
            for t in np.unique(sel[:, 2]):
                bins = sel[sel[:, 2] == t][:, 0]
                win = xp64[b, t * HOP:t * HOP + L_in]
                re[bins, b, t] = W64[bins] @ win
                im[bins, b, t] = W64[bins + N_BINS] @ win
        mag = np.sqrt(re * re + im * im)

    ref = max(mag.max(), AMIN)
    log_spec = 10.0 * np.log10(np.maximum(mag, AMIN)) - 10.0 * np.log10(ref)
    log_spec = np.maximum(log_spec, log_spec.max() - TOP_DB)
    return np.ascontiguousarray(log_spec.transpose(1, 2, 0)).astype(np.float32)



# revision 2
# speedup vs baseline: 2.8565x; 2.8565x over previous
"""CQT layer kernel for Trainium2 (8 NeuronCores, SPMD) — block-sparse.

The strided conv (hop 128 == PE contraction tile) is a chunked matmul:
  out[c, b, t] = sum_k  W[c, 128k:128k+128] . xT_b[:, t+k]
The CQT filterbank is ~18% dense: per-bin kernel length Nk = Q*SR/freq
shrinks geometrically with bin index, and every kernel is centered in the
common window.  Channels are sorted by length and grouped into 128-wide
blocks (64 bins x {re,im}); each block only touches the chunks its longest
bin covers, so the full job is ~1023 (block, chunk) matmuls instead of the
dense 8.25*499 = 4116.

All cores run ONE program: 6 fixed-length segments [66,33,17,8,4,2] = 130
matmul slots, each segment accumulating into its own PSUM bank and writing
its own [128, 348] partial.  Which (block, chunk-run) a slot computes is
pure DATA: the host packs that slot's weight chunks and the matching
shifted window of x columns, and sums the partials afterwards.  A runtime
first-fit solver assigns block runs to the 8x6 slot pool.

Magnitude + power_to_db run on host, with an exact fp64 recompute of the
few near-silent bins where fp16 matmul error would be audible in dB.

Self-contained: only needs numpy + the concourse toolchain at /opt/trn_rl_repo.
"""
import os
import sys

sys.path.insert(0, "/opt/trn_rl_repo")
import numpy as np

# ---- problem constants (hardcoded from the CQT layer spec) ----
B = 2
AUDIO_LEN = 22016
N_BINS = 528
NCH = 2 * N_BINS          # 1056 conv channels (re, im)
HOP = 128
FRAMES = 173
AMIN = 1e-10
TOP_DB = 80.0

K = 128                   # PE contraction tile == HOP
NCHUNK = 499              # ceil(L / 128); holds for L in [63745, 63872]
LPAD = NCHUNK * K         # 63872
NT = 174                  # frames padded to even
BNT = B * NT              # 348 moving columns per matmul
NROW = NCHUNK + NT - 1    # 672 columns of xT per batch
N_CORES = 8
BPB = 64                  # bins per 128-channel block
NBLK = 9                  # 8 full blocks + 32-channel tail block (zero-padded)

SEGS = [int(v) for v in os.environ.get("CQT_SEGS", "66,33,17,8,4,2").split(",")]
NSEG = len(SEGS)
NSLOT = sum(SEGS)         # matmul slots per core (130)
XWIN = [s + NT - 1 for s in SEGS]          # x columns per segment window
XOFF = np.concatenate([[0], np.cumsum(XWIN)])
XCOLS = int(XOFF[-1])                      # total x columns per core
SOFF = np.concatenate([[0], np.cumsum(SEGS)])  # slot offset per segment

DTYPE = os.environ.get("CQT_DTYPE", "float16")  # float16 | float32r
_CONV_EPS = {"float16": 1e-3, "float32r": 5e-4, "bfloat16": 5e-3}
DB_ERR_TARGET = 0.02      # refine bins whose worst-case dB error exceeds this

_prog_cache = {}


def _np_cast(a):
    if DTYPE == "float16":
        return a.astype(np.float16)
    if DTYPE == "bfloat16":
        import ml_dtypes
        return a.astype(ml_dtypes.bfloat16)
    return a  # float32r: raw fp32 bits


def _build_program():
    from concourse import bacc, mybir
    from concourse.tile import TileContext

    dt = mybir.dt
    DT = getattr(dt, DTYPE)

    nc = bacc.Bacc(None, target_bir_lowering=False)
    xs_p = nc.declare_dram_parameter("xs", [K, XCOLS * B], DT, isOutput=False)
    wm_p = nc.declare_dram_parameter("wm", [K, NSLOT * K], DT, isOutput=False)
    om_p = nc.declare_dram_parameter("om", [K, NSEG * BNT], dt.float32, isOutput=True)

    # weight DMA groups (in slots): small first so the PE starts early
    groups = []
    k0 = 0
    ramp = [int(v) for v in os.environ.get("CQT_RAMP", "2,4,8").split(",") if v]
    for g in ramp:
        groups.append((k0, g))
        k0 += g
    GROUP = int(os.environ.get("CQT_GROUP", "16"))
    while k0 < NSLOT:
        cnt = min(GROUP, NSLOT - k0)
        groups.append((k0, cnt))
        k0 += cnt
    N_WARM = int(os.environ.get("CQT_WARM", "8"))
    X0 = XWIN[0] * B          # first segment's x window, needed immediately

    with TileContext(nc) as tc:
        with (
            tc.tile_pool(name="stat", bufs=1) as stat,
            tc.tile_pool(name="opool", bufs=1) as opool,
            tc.tile_pool(name="ps", bufs=1, space="PSUM") as ps,
        ):
            # tiny PE warm-up matmuls: no DMA dependency, keep HAM busy
            # during the input-DMA window without delaying real work
            warm_sb = stat.tile([K, 16], DT)
            nc.gpsimd.memset(warm_sb[:], 0.0)
            ps_warm = ps.tile([16, 16], dt.float32)
            for _ in range(N_WARM):
                nc.tensor.matmul(ps_warm[:], warm_sb[:, :16], warm_sb[:, :16],
                                 start=True, stop=True)

            xs_sb = stat.tile([K, XCOLS * B], DT)
            nc.sync.dma_start(xs_sb[:, :X0], xs_p[:, :X0])
            wm_sb = stat.tile([K, NSLOT * K], DT)
            for gi, (g0, cnt) in enumerate(groups):
                nc.sync.dma_start(
                    wm_sb[:, g0 * K:(g0 + cnt) * K],
                    wm_p[:, g0 * K:(g0 + cnt) * K],
                )
                if gi == 0:
                    nc.sync.dma_start(xs_sb[:, X0:], xs_p[:, X0:])

            x3 = xs_sb[:].rearrange("p (t b) -> p t b", b=B)

            om_sb = opool.tile([K, NSEG * BNT], dt.float32)
            for s in range(NSEG):
                ps_s = ps.tile([K, BNT], dt.float32, tag=f"ps{s}", name=f"ps{s}")
                p3 = ps_s[:].rearrange("p (t b) -> p t b", b=B)
                for j in range(SEGS[s]):
                    nc.tensor.matmul(
                        p3,
                        wm_sb[:, (SOFF[s] + j) * K:(SOFF[s] + j + 1) * K],
                        x3[:, XOFF[s] + j:XOFF[s] + j + NT, :],
                        start=(j == 0),
                        stop=(j == SEGS[s] - 1),
                    )
                sl = slice(s * BNT, (s + 1) * BNT)
                nc.vector.tensor_copy(om_sb[:, sl], ps_s[:])
                nc.sync.dma_start(om_p[:, sl], om_sb[:, sl])

    nc.finalize()
    return nc


def _solve_assignment(block_ranges):
    """Assign each block's chunk range to fixed-size slots.

    Returns per-core slot tables: assign[core][seg] = (block, k0) or None.
    Every slot of segment s covers exactly SEGS[s] consecutive chunks
    starting at k0 (chunks past the block range are zero-padded weights).
    """
    avail = {s: list(range(N_CORES)) for s in set(SEGS)}
    # (core, seg) slots grouped by size; seg index recovered per core below
    slot_of = [[None] * NSEG for _ in range(N_CORES)]
    seg_by_size = {}
    for s, ln in enumerate(SEGS):
        seg_by_size.setdefault(ln, []).append(s)
    # per size, a pool of (core, seg) pairs
    pool = {ln: [(c, s) for c in range(N_CORES) for s in seg_by_size[ln]]
            for ln in seg_by_size}
    sizes = sorted(pool, reverse=True)

    order = sorted(range(len(block_ranges)),
                   key=lambda b: block_ranges[b][0] - block_ranges[b][1])
    for b in order:
        c0, c1 = block_ranges[b]
        rem = c1 - c0
        k = c0
        while rem > 0:
            pick = None
            for ln in sizes:
                if ln <= rem and pool[ln]:
                    pick = ln
                    break
            if pick is None:  # pad with the smallest available slot
                for ln in reversed(sizes):
                    if pool[ln]:
                        pick = ln
                        break
            if pick is None:
                raise RuntimeError("slot pool exhausted; adjust CQT_SEGS")
            core, seg = pool[pick].pop()
            slot_of[core][seg] = (b, k)
            k += pick
            rem -= pick
    return slot_of


LAST_RESULTS = None


def kernel(y, kern_r, kern_i):
    global LAST_RESULTS
    from concourse.bass_utils import run_bass_kernel_spmd

    y = np.asarray(y, dtype=np.float32)
    kern_r = np.asarray(kern_r, dtype=np.float32)
    kern_i = np.asarray(kern_i, dtype=np.float32)

    # ---- host prep: channel sort + per-block chunk ranges ----
    L_in = kern_r.shape[1]
    pad = L_in // 2
    assert (NCHUNK - 1) * K < L_in <= LPAD, L_in
    # channels interleaved (re0, im0, re1, im1, ...) so a 128-channel block
    # holds 64 consecutive bins and their lengths stay as uniform as possible
    Ws = np.empty((NCH, L_in), np.float32)
    Ws[0::2] = kern_r
    Ws[1::2] = kern_i
    nz = np.abs(Ws) > 0
    first = nz.argmax(axis=1)
    last = L_in - nz[:, ::-1].argmax(axis=1)          # one past last nonzero
    block_ranges = []
    for g in range(NBLK):
        lo = int(first[2 * BPB * g:2 * BPB * (g + 1)].min()) // K
        hi = -(-int(last[2 * BPB * g:2 * BPB * (g + 1)].max()) // K)
        block_ranges.append((lo, hi))
    assign = _solve_assignment(block_ranges)

    Wp = np.zeros((NCH, LPAD), np.float32)
    Wp[:, :L_in] = Ws
    Wk = Wp.reshape(NCH, NCHUNK, K)                   # [c_sorted, k, l]

    # ---- host prep: audio -> xT [128, per-batch 672 cols] ----
    x_pad = np.zeros((B, NROW * K), np.float32)
    x_pad[:, pad:pad + AUDIO_LEN] = y
    xT = np.ascontiguousarray(x_pad.reshape(B, NROW, K).transpose(0, 2, 1))

    in_maps = []
    for i in range(N_CORES):
        wm = np.zeros((K, NSLOT, K), np.float32)      # [l, slot, ch]
        xs = np.zeros((K, XCOLS, B), np.float32)      # [l, col, b]
        for s in range(NSEG):
            a = assign[i][s]
            if a is None:
                continue
            blk, k0 = a
            ch0 = 128 * blk
            ch1 = min(ch0 + 128, NCH)
            kl0, kh0 = k0, min(k0 + SEGS[s], NCHUNK)
            if kh0 > kl0:
                # weights: [ch, chunk, l] -> [l, slot, ch]
                wm[:, SOFF[s] + 0:SOFF[s] + kh0 - kl0, :ch1 - ch0] = \
                    Wk[ch0:ch1, kl0:kh0].transpose(2, 1, 0)
            g0, g1 = k0, min(k0 + XWIN[s], NROW)
            if g1 > g0:
                xs[:, XOFF[s]:XOFF[s] + g1 - g0, :] = \
                    xT[:, :, g0:g1].transpose(1, 2, 0)
        in_maps.append({
            "xs": _np_cast(np.ascontiguousarray(xs.reshape(K, XCOLS * B))),
            "wm": _np_cast(np.ascontiguousarray(wm.reshape(K, NSLOT * K))),
        })

    if DTYPE not in _prog_cache:
        _prog_cache[DTYPE] = _build_program()
    nc = _prog_cache[DTYPE]

    LAST_RESULTS = run_bass_kernel_spmd(
        nc, in_maps, list(range(N_CORES)),
        trace=bool(os.environ.get("CQT_TRACE")),
    )
    results = LAST_RESULTS.results

    # ---- host post: sum partials per block, un-permute, magnitude, dB ----
    conv_s = np.zeros((NCH, B, FRAMES), np.float64)   # sorted channel order
    for i in range(N_CORES):
        om = results[i]["om"].reshape(K, NSEG, NT, B)
        for s in range(NSEG):
            a = assign[i][s]
            if a is None:
                continue
            blk, _ = a
            ch0 = 128 * blk
            ch1 = min(ch0 + 128, NCH)
            conv_s[ch0:ch1] += om[:ch1 - ch0, s, :FRAMES, :].transpose(0, 2, 1)

    re = conv_s[0::2]                                  # [528, B, 173]
    im = conv_s[1::2]
    mag = np.sqrt(re * re + im * im)

    # ---- host refinement: exact recompute of near-silent bins ----
    conv_rms = float(np.sqrt(np.mean(mag * mag)))
    err_abs = _CONV_EPS.get(DTYPE, 1e-3) * conv_rms
    thresh = 4.343 * err_abs / DB_ERR_TARGET
    fix = np.argwhere(mag < thresh)                    # rows: (bin, b, t)
    if len(fix):
        xp64 = x_pad.astype(np.float64)
        for b in range(B):
            sel = fix[fix[:, 1] == b]
            if not len(sel):
                continue
            for t in np.unique(sel[:, 2]):
                bins = sel[sel[:, 2] == t][:, 0]
                win = xp64[b, t * HOP:t * HOP + L_in]
                re[bins, b, t] = kern_r[bins].astype(np.float64) @ win
                im[bins, b, t] = kern_i[bins].astype(np.float64) @ win
        mag = np.sqrt(re * re + im * im)

    ref = max(mag.max(), AMIN)
    log_spec = 10.0 * np.log10(np.maximum(mag, AMIN)) - 10.0 * np.log10(ref)
    log_spec = np.maximum(log_spec, log_spec.max() - TOP_DB)
    return np.ascontiguousarray(log_spec.transpose(1, 2, 0)).astype(np.float32)


# revision 4
# speedup vs baseline: 2.9350x; 1.0275x over previous
"""CQT layer kernel for Trainium2 (8 NeuronCores, SPMD) — block-sparse.

The strided conv (hop 128 == PE contraction tile) is a chunked matmul:
  out[c, b, t] = sum_k  W[c, 128k:128k+128] . xT_b[:, t+k]
The CQT filterbank is ~18% dense: per-bin kernel length Nk = Q*SR/freq
shrinks geometrically with bin index, and every kernel is centered in the
common window.  Channels are sorted by length and grouped into 128-wide
blocks (64 bins x {re,im}); each block only touches the chunks its longest
bin covers, so the full job is ~1023 (block, chunk) matmuls instead of the
dense 8.25*499 = 4116.

All cores run ONE program: 6 fixed-length segments [66,33,17,8,4,2] = 130
matmul slots, each segment accumulating into its own PSUM bank and writing
its own [128, 348] partial.  Which (block, chunk-run) a slot computes is
pure DATA: the host packs that slot's weight chunks and the matching
shifted window of x columns, and sums the partials afterwards.  A runtime
first-fit solver assigns block runs to the 8x6 slot pool.

Magnitude + power_to_db run on host, with an exact fp64 recompute of the
few near-silent bins where fp16 matmul error would be audible in dB.

Self-contained: only needs numpy + the concourse toolchain at /opt/trn_rl_repo.
"""
import os
import sys

sys.path.insert(0, "/opt/trn_rl_repo")
import numpy as np

# ---- problem constants (hardcoded from the CQT layer spec) ----
B = 2
AUDIO_LEN = 22016
N_BINS = 528
NCH = 2 * N_BINS          # 1056 conv channels (re, im)
HOP = 128
FRAMES = 173
AMIN = 1e-10
TOP_DB = 80.0

K = 128                   # PE contraction tile == HOP
NCHUNK = 499              # ceil(L / 128); holds for L in [63745, 63872]
LPAD = NCHUNK * K         # 63872
NT = 174                  # frames padded to even
BNT = B * NT              # 348 moving columns per matmul
NROW = NCHUNK + NT - 1    # 672 columns of xT per batch
N_CORES = 8
BPB = 64                  # bins per 128-channel block
NBLK = 9                  # 8 full blocks + 32-channel tail block (zero-padded)

SEGS = [int(v) for v in os.environ.get("CQT_SEGS", "66,33,17,8,4,2").split(",")]
NSEG = len(SEGS)
NSLOT = sum(SEGS)         # matmul slots per core (130)
XWIN = [s + NT - 1 for s in SEGS]          # x columns per segment window
XOFF = np.concatenate([[0], np.cumsum(XWIN)])
XCOLS = int(XOFF[-1])                      # total x columns per core
SOFF = np.concatenate([[0], np.cumsum(SEGS)])  # slot offset per segment

DTYPE = os.environ.get("CQT_DTYPE", "float16")  # float16 | float32r
_CONV_EPS = {"float16": 1e-3, "float32r": 5e-4, "bfloat16": 5e-3}
DB_ERR_TARGET = 0.02      # refine bins whose worst-case dB error exceeds this

_prog_cache = {}


def _np_cast(a):
    if DTYPE == "float16":
        return a.astype(np.float16)
    if DTYPE == "bfloat16":
        import ml_dtypes
        return a.astype(ml_dtypes.bfloat16)
    return a  # float32r: raw fp32 bits


def _build_program():
    from concourse import bacc, mybir
    from concourse.tile import TileContext

    dt = mybir.dt
    DT = getattr(dt, DTYPE)

    nc = bacc.Bacc(None, target_bir_lowering=False)
    xs_p = nc.declare_dram_parameter("xs", [K, XCOLS * B], DT, isOutput=False)
    wm_p = nc.declare_dram_parameter("wm", [K, NSLOT * K], DT, isOutput=False)
    om_p = nc.declare_dram_parameter("om", [K, NSEG * BNT], dt.float32, isOutput=True)

    # weight DMA groups (in slots): small first so the PE starts early
    groups = []
    k0 = 0
    ramp = [int(v) for v in os.environ.get("CQT_RAMP", "4,8,16").split(",") if v]
    for g in ramp:
        groups.append((k0, g))
        k0 += g
    GROUP = int(os.environ.get("CQT_GROUP", "32"))
    while k0 < NSLOT:
        cnt = min(GROUP, NSLOT - k0)
        groups.append((k0, cnt))
        k0 += cnt
    XREST_AFTER = int(os.environ.get("CQT_XREST", "3"))
    N_WARM = int(os.environ.get("CQT_WARM", "8"))
    X0 = XWIN[0] * B          # first segment's x window, needed immediately

    with TileContext(nc) as tc:
        with (
            tc.tile_pool(name="stat", bufs=1) as stat,
            tc.tile_pool(name="opool", bufs=1) as opool,
            tc.tile_pool(name="ps", bufs=1, space="PSUM") as ps,
        ):
            # tiny PE warm-up matmuls: no DMA dependency, keep HAM busy
            # during the input-DMA window without delaying real work
            warm_sb = stat.tile([K, 16], DT)
            nc.gpsimd.memset(warm_sb[:], 0.0)
            ps_warm = ps.tile([16, 16], dt.float32)
            for _ in range(N_WARM):
                nc.tensor.matmul(ps_warm[:], warm_sb[:, :16], warm_sb[:, :16],
                                 start=True, stop=True)

            xs_sb = stat.tile([K, XCOLS * B], DT)
            nc.sync.dma_start(xs_sb[:, :X0], xs_p[:, :X0])
            wm_sb = stat.tile([K, NSLOT * K], DT)
            for gi, (g0, cnt) in enumerate(groups):
                nc.sync.dma_start(
                    wm_sb[:, g0 * K:(g0 + cnt) * K],
                    wm_p[:, g0 * K:(g0 + cnt) * K],
                )
                if gi == XREST_AFTER:
                    nc.sync.dma_start(xs_sb[:, X0:], xs_p[:, X0:])

            x3 = xs_sb[:].rearrange("p (t b) -> p t b", b=B)

            om_sb = opool.tile([K, NSEG * BNT], dt.float32)
            for s in range(NSEG):
                ps_s = ps.tile([K, BNT], dt.float32, tag=f"ps{s}", name=f"ps{s}")
                p3 = ps_s[:].rearrange("p (t b) -> p t b", b=B)
                for j in range(SEGS[s]):
                    nc.tensor.matmul(
                        p3,
                        wm_sb[:, (SOFF[s] + j) * K:(SOFF[s] + j + 1) * K],
                        x3[:, XOFF[s] + j:XOFF[s] + j + NT, :],
                        start=(j == 0),
                        stop=(j == SEGS[s] - 1),
                    )
                sl = slice(s * BNT, (s + 1) * BNT)
                nc.vector.tensor_copy(om_sb[:, sl], ps_s[:])
                nc.sync.dma_start(om_p[:, sl], om_sb[:, sl])

    nc.finalize()
    return nc


def _solve_assignment(block_ranges):
    """Assign each block's chunk range to fixed-size slots.

    Returns per-core slot tables: assign[core][seg] = (block, k0) or None.
    Every slot of segment s covers exactly SEGS[s] consecutive chunks
    starting at k0 (chunks past the block range are zero-padded weights).
    """
    avail = {s: list(range(N_CORES)) for s in set(SEGS)}
    # (core, seg) slots grouped by size; seg index recovered per core below
    slot_of = [[None] * NSEG for _ in range(N_CORES)]
    seg_by_size = {}
    for s, ln in enumerate(SEGS):
        seg_by_size.setdefault(ln, []).append(s)
    # per size, a pool of (core, seg) pairs
    pool = {ln: [(c, s) for c in range(N_CORES) for s in seg_by_size[ln]]
            for ln in seg_by_size}
    sizes = sorted(pool, reverse=True)

    order = sorted(range(len(block_ranges)),
                   key=lambda b: block_ranges[b][0] - block_ranges[b][1])
    for b in order:
        c0, c1 = block_ranges[b]
        rem = c1 - c0
        k = c0
        while rem > 0:
            pick = None
            for ln in sizes:
                if ln <= rem and pool[ln]:
                    pick = ln
                    break
            if pick is None:  # pad with the smallest available slot
                for ln in reversed(sizes):
                    if pool[ln]:
                        pick = ln
                        break
            if pick is None:
                raise RuntimeError("slot pool exhausted; adjust CQT_SEGS")
            core, seg = pool[pick].pop()
            slot_of[core][seg] = (b, k)
            k += pick
            rem -= pick
    return slot_of


LAST_RESULTS = None


def kernel(y, kern_r, kern_i):
    global LAST_RESULTS
    from concourse.bass_utils import run_bass_kernel_spmd

    y = np.asarray(y, dtype=np.float32)
    kern_r = np.asarray(kern_r, dtype=np.float32)
    kern_i = np.asarray(kern_i, dtype=np.float32)

    # ---- host prep: channel sort + per-block chunk ranges ----
    L_in = kern_r.shape[1]
    pad = L_in // 2
    assert (NCHUNK - 1) * K < L_in <= LPAD, L_in
    # channels interleaved (re0, im0, re1, im1, ...) so a 128-channel block
    # holds 64 consecutive bins and their lengths stay as uniform as possible
    Ws = np.empty((NCH, L_in), np.float32)
    Ws[0::2] = kern_r
    Ws[1::2] = kern_i
    nz = np.abs(Ws) > 0
    first = nz.argmax(axis=1)
    last = L_in - nz[:, ::-1].argmax(axis=1)          # one past last nonzero
    block_ranges = []
    for g in range(NBLK):
        lo = int(first[2 * BPB * g:2 * BPB * (g + 1)].min()) // K
        hi = -(-int(last[2 * BPB * g:2 * BPB * (g + 1)].max()) // K)
        block_ranges.append((lo, hi))
    assign = _solve_assignment(block_ranges)

    Wp = np.zeros((NCH, LPAD), np.float32)
    Wp[:, :L_in] = Ws
    Wk = Wp.reshape(NCH, NCHUNK, K)                   # [c_sorted, k, l]

    # ---- host prep: audio -> xT [128, per-batch 672 cols] ----
    x_pad = np.zeros((B, NROW * K), np.float32)
    x_pad[:, pad:pad + AUDIO_LEN] = y
    xT = np.ascontiguousarray(x_pad.reshape(B, NROW, K).transpose(0, 2, 1))

    in_maps = []
    for i in range(N_CORES):
        wm = np.zeros((K, NSLOT, K), np.float32)      # [l, slot, ch]
        xs = np.zeros((K, XCOLS, B), np.float32)      # [l, col, b]
        for s in range(NSEG):
            a = assign[i][s]
            if a is None:
                continue
            blk, k0 = a
            ch0 = 128 * blk
            ch1 = min(ch0 + 128, NCH)
            kl0, kh0 = k0, min(k0 + SEGS[s], NCHUNK)
            if kh0 > kl0:
                # weights: [ch, chunk, l] -> [l, slot, ch]
                wm[:, SOFF[s] + 0:SOFF[s] + kh0 - kl0, :ch1 - ch0] = \
                    Wk[ch0:ch1, kl0:kh0].transpose(2, 1, 0)
            g0, g1 = k0, min(k0 + XWIN[s], NROW)
            if g1 > g0:
                xs[:, XOFF[s]:XOFF[s] + g1 - g0, :] = \
                    xT[:, :, g0:g1].transpose(1, 2, 0)
        in_maps.append({
            "xs": _np_cast(np.ascontiguousarray(xs.reshape(K, XCOLS * B))),
            "wm": _np_cast(np.ascontiguousarray(wm.reshape(K, NSLOT * K))),
        })

    if DTYPE not in _prog_cache:
        _prog_cache[DTYPE] = _build_program()
    nc = _prog_cache[DTYPE]

    LAST_RESULTS = run_bass_kernel_spmd(
        nc, in_maps, list(range(N_CORES)),
        trace=bool(os.environ.get("CQT_TRACE")),
    )
    results = LAST_RESULTS.results

    # ---- host post: sum partials per block, un-permute, magnitude, dB ----
    conv_s = np.zeros((NCH, B, FRAMES), np.float64)   # sorted channel order
    for i in range(N_CORES):
        om = results[i]["om"].reshape(K, NSEG, NT, B)
        for s in range(NSEG):
            a = assign[i][s]
            if a is None:
                continue
            blk, _ = a
            ch0 = 128 * blk
            ch1 = min(ch0 + 128, NCH)
            conv_s[ch0:ch1] += om[:ch1 - ch0, s, :FRAMES, :].transpose(0, 2, 1)

    re = conv_s[0::2]                                  # [528, B, 173]
    im = conv_s[1::2]
    mag = np.sqrt(re * re + im * im)

    # ---- host refinement: exact recompute of near-silent bins ----
    conv_rms = float(np.sqrt(np.mean(mag * mag)))
    err_abs = _CONV_EPS.get(DTYPE, 1e-3) * conv_rms
    thresh = 4.343 * err_abs / DB_ERR_TARGET
    fix = np.argwhere(mag < thresh)                    # rows: (bin, b, t)
    if len(fix):
        xp64 = x_pad.astype(np.float64)
        for b in range(B):
            sel = fix[fix[:, 1] == b]
            if not len(sel):
                continue
            for t in np.unique(sel[:, 2]):
                bins = sel[sel[:, 2] == t][:, 0]
                win = xp64[b, t * HOP:t * HOP + L_in]
                re[bins, b, t] = kern_r[bins].astype(np.float64) @ win
                im[bins, b, t] = kern_i[bins].astype(np.float64) @ win
        mag = np.sqrt(re * re + im * im)

    ref = max(mag.max(), AMIN)
    log_spec = 10.0 * np.log10(np.maximum(mag, AMIN)) - 10.0 * np.log10(ref)
    log_spec = np.maximum(log_spec, log_spec.max() - TOP_DB)
    return np.ascontiguousarray(log_spec.transpose(1, 2, 0)).astype(np.float32)


# revision 6
# speedup vs baseline: 3.0128x; 1.0265x over previous
"""CQT layer kernel for Trainium2 (8 NeuronCores, SPMD) — block-sparse.

The strided conv (hop 128 == PE contraction tile) is a chunked matmul:
  out[c, b, t] = sum_k  W[c, 128k:128k+128] . xT_b[:, t+k]
The CQT filterbank is ~18% dense: per-bin kernel length Nk = Q*SR/freq
shrinks geometrically with bin index, and every kernel is centered in the
common window.  Channels are sorted by length and grouped into 128-wide
blocks (64 bins x {re,im}); each block only touches the chunks its longest
bin covers, so the full job is ~1023 (block, chunk) matmuls instead of the
dense 8.25*499 = 4116.

All cores run ONE program: 6 fixed-length segments [66,33,17,8,4,2] = 130
matmul slots, each segment accumulating into its own PSUM bank and writing
its own [128, 348] partial.  Which (block, chunk-run) a slot computes is
pure DATA: the host packs that slot's weight chunks and the matching
shifted window of x columns, and sums the partials afterwards.  A runtime
first-fit solver assigns block runs to the 8x6 slot pool.

Magnitude + power_to_db run on host, with an exact fp64 recompute of the
few near-silent bins where fp16 matmul error would be audible in dB.

Self-contained: only needs numpy + the concourse toolchain at /opt/trn_rl_repo.
"""
import os
import sys

sys.path.insert(0, "/opt/trn_rl_repo")
import numpy as np

# ---- problem constants (hardcoded from the CQT layer spec) ----
B = 2
AUDIO_LEN = 22016
N_BINS = 528
NCH = 2 * N_BINS          # 1056 conv channels (re, im)
HOP = 128
FRAMES = 173
AMIN = 1e-10
TOP_DB = 80.0

K = 128                   # PE contraction tile == HOP
NCHUNK = 499              # ceil(L / 128); holds for L in [63745, 63872]
LPAD = NCHUNK * K         # 63872
NT = 174                  # frames padded to even
BNT = B * NT              # 348 moving columns per matmul
NROW = NCHUNK + NT - 1    # 672 columns of xT per batch
N_CORES = 8
BPB = 64                  # bins per 128-channel block
NBLK = 9                  # 8 full blocks + 32-channel tail block (zero-padded)

SEGS = [int(v) for v in os.environ.get("CQT_SEGS", "66,33,17,8,4,2").split(",")]
NSEG = len(SEGS)
NSLOT = sum(SEGS)         # matmul slots per core (130)
XWIN = [s + NT - 1 for s in SEGS]          # x columns per segment window
XOFF = np.concatenate([[0], np.cumsum(XWIN)])
XCOLS = int(XOFF[-1])                      # total x columns per core
SOFF = np.concatenate([[0], np.cumsum(SEGS)])  # slot offset per segment

DTYPE = os.environ.get("CQT_DTYPE", "float16")  # float16 | float32r
_CONV_EPS = {"float16": 1e-3, "float32r": 5e-4, "bfloat16": 5e-3}
DB_ERR_TARGET = 0.02      # refine bins whose worst-case dB error exceeds this

_prog_cache = {}


def _np_cast(a):
    if DTYPE == "float16":
        return a.astype(np.float16)
    if DTYPE == "bfloat16":
        import ml_dtypes
        return a.astype(ml_dtypes.bfloat16)
    return a  # float32r: raw fp32 bits


def _build_program():
    from concourse import bacc, mybir
    from concourse.tile import TileContext

    dt = mybir.dt
    DT = getattr(dt, DTYPE)

    nc = bacc.Bacc(None, target_bir_lowering=False)
    xs_p = nc.declare_dram_parameter("xs", [K, XCOLS * B], DT, isOutput=False)
    wm_p = nc.declare_dram_parameter("wm", [K, NSLOT * K], DT, isOutput=False)
    om_p = nc.declare_dram_parameter("om", [K, NSEG * BNT], dt.float32, isOutput=True)

    # weight DMA groups (in slots): small first so the PE starts early
    groups = []
    k0 = 0
    ramp = [int(v) for v in os.environ.get("CQT_RAMP", "4,8,16").split(",") if v]
    for g in ramp:
        groups.append((k0, g))
        k0 += g
    GROUP = int(os.environ.get("CQT_GROUP", "32"))
    while k0 < NSLOT:
        cnt = min(GROUP, NSLOT - k0)
        groups.append((k0, cnt))
        k0 += cnt
    XREST_AFTER = int(os.environ.get("CQT_XREST", "3"))
    # PE warm-up: HAM un-throttles (1.2 -> 2.4 GHz) only after ~3.4us of
    # sustained PE activity, and the input-DMA ramp stalls the real matmul
    # stream early, resetting the window.  Fill the ramp window with dummy
    # matmuls so the clock is warm when the real stream starts.
    N_WARM = int(os.environ.get("CQT_WARM", "24"))
    WARM_N = int(os.environ.get("CQT_WARMN", "128"))
    X0 = XWIN[0] * B          # first segment's x window, needed immediately

    with TileContext(nc) as tc:
        with (
            tc.tile_pool(name="stat", bufs=1) as stat,
            tc.tile_pool(name="opool", bufs=1) as opool,
            tc.tile_pool(name="ps", bufs=1, space="PSUM") as ps,
        ):
            warm_sb = stat.tile([K, WARM_N], DT)
            nc.gpsimd.memset(warm_sb[:], 0.0)
            ps_warm = ps.tile([16, WARM_N], dt.float32)
            for _ in range(N_WARM):
                nc.tensor.matmul(ps_warm[:], warm_sb[:, :16], warm_sb[:],
                                 start=True, stop=True)

            xs_sb = stat.tile([K, XCOLS * B], DT)
            nc.sync.dma_start(xs_sb[:, :X0], xs_p[:, :X0])
            wm_sb = stat.tile([K, NSLOT * K], DT)
            for gi, (g0, cnt) in enumerate(groups):
                nc.sync.dma_start(
                    wm_sb[:, g0 * K:(g0 + cnt) * K],
                    wm_p[:, g0 * K:(g0 + cnt) * K],
                )
                if gi == XREST_AFTER:
                    nc.sync.dma_start(xs_sb[:, X0:], xs_p[:, X0:])

            x3 = xs_sb[:].rearrange("p (t b) -> p t b", b=B)

            om_sb = opool.tile([K, NSEG * BNT], dt.float32)
            for s in range(NSEG):
                ps_s = ps.tile([K, BNT], dt.float32, tag=f"ps{s}", name=f"ps{s}")
                p3 = ps_s[:].rearrange("p (t b) -> p t b", b=B)
                for j in range(SEGS[s]):
                    nc.tensor.matmul(
                        p3,
                        wm_sb[:, (SOFF[s] + j) * K:(SOFF[s] + j + 1) * K],
                        x3[:, XOFF[s] + j:XOFF[s] + j + NT, :],
                        start=(j == 0),
                        stop=(j == SEGS[s] - 1),
                    )
                sl = slice(s * BNT, (s + 1) * BNT)
                nc.vector.tensor_copy(om_sb[:, sl], ps_s[:])
                nc.sync.dma_start(om_p[:, sl], om_sb[:, sl])

    nc.finalize()
    return nc


def _solve_assignment(block_ranges):
    """Assign each block's chunk range to fixed-size slots.

    Returns per-core slot tables: assign[core][seg] = (block, k0) or None.
    Every slot of segment s covers exactly SEGS[s] consecutive chunks
    starting at k0 (chunks past the block range are zero-padded weights).
    """
    avail = {s: list(range(N_CORES)) for s in set(SEGS)}
    # (core, seg) slots grouped by size; seg index recovered per core below
    slot_of = [[None] * NSEG for _ in range(N_CORES)]
    seg_by_size = {}
    for s, ln in enumerate(SEGS):
        seg_by_size.setdefault(ln, []).append(s)
    # per size, a pool of (core, seg) pairs
    pool = {ln: [(c, s) for c in range(N_CORES) for s in seg_by_size[ln]]
            for ln in seg_by_size}
    sizes = sorted(pool, reverse=True)

    order = sorted(range(len(block_ranges)),
                   key=lambda b: block_ranges[b][0] - block_ranges[b][1])
    for b in order:
        c0, c1 = block_ranges[b]
        rem = c1 - c0
        k = c0
        while rem > 0:
            pick = None
            for ln in sizes:
                if ln <= rem and pool[ln]:
                    pick = ln
                    break
            if pick is None:  # pad with the smallest available slot
                for ln in reversed(sizes):
                    if pool[ln]:
                        pick = ln
                        break
            if pick is None:
                raise RuntimeError("slot pool exhausted; adjust CQT_SEGS")
            core, seg = pool[pick].pop()
            slot_of[core][seg] = (b, k)
            k += pick
            rem -= pick
    return slot_of


LAST_RESULTS = None


def kernel(y, kern_r, kern_i):
    global LAST_RESULTS
    from concourse.bass_utils import run_bass_kernel_spmd

    y = np.asarray(y, dtype=np.float32)
    kern_r = np.asarray(kern_r, dtype=np.float32)
    kern_i = np.asarray(kern_i, dtype=np.float32)

    # ---- host prep: channel sort + per-block chunk ranges ----
    L_in = kern_r.shape[1]
    pad = L_in // 2
    assert (NCHUNK - 1) * K < L_in <= LPAD, L_in
    # channels interleaved (re0, im0, re1, im1, ...) so a 128-channel block
    # holds 64 consecutive bins and their lengths stay as uniform as possible
    Ws = np.empty((NCH, L_in), np.float32)
    Ws[0::2] = kern_r
    Ws[1::2] = kern_i
    nz = np.abs(Ws) > 0
    first = nz.argmax(axis=1)
    last = L_in - nz[:, ::-1].argmax(axis=1)          # one past last nonzero
    block_ranges = []
    for g in range(NBLK):
        lo = int(first[2 * BPB * g:2 * BPB * (g + 1)].min()) // K
        hi = -(-int(last[2 * BPB * g:2 * BPB * (g + 1)].max()) // K)
        block_ranges.append((lo, hi))
    assign = _solve_assignment(block_ranges)

    Wp = np.zeros((NCH, LPAD), np.float32)
    Wp[:, :L_in] = Ws
    Wk = Wp.reshape(NCH, NCHUNK, K)                   # [c_sorted, k, l]

    # ---- host prep: audio -> xT [128, per-batch 672 cols] ----
    x_pad = np.zeros((B, NROW * K), np.float32)
    x_pad[:, pad:pad + AUDIO_LEN] = y
    xT = np.ascontiguousarray(x_pad.reshape(B, NROW, K).transpose(0, 2, 1))

    in_maps = []
    for i in range(N_CORES):
        wm = np.zeros((K, NSLOT, K), np.float32)      # [l, slot, ch]
        xs = np.zeros((K, XCOLS, B), np.float32)      # [l, col, b]
        for s in range(NSEG):
            a = assign[i][s]
            if a is None:
                continue
            blk, k0 = a
            ch0 = 128 * blk
            ch1 = min(ch0 + 128, NCH)
            kl0, kh0 = k0, min(k0 + SEGS[s], NCHUNK)
            if kh0 > kl0:
                # weights: [ch, chunk, l] -> [l, slot, ch]
                wm[:, SOFF[s] + 0:SOFF[s] + kh0 - kl0, :ch1 - ch0] = \
                    Wk[ch0:ch1, kl0:kh0].transpose(2, 1, 0)
            g0, g1 = k0, min(k0 + XWIN[s], NROW)
            if g1 > g0:
                xs[:, XOFF[s]:XOFF[s] + g1 - g0, :] = \
                    xT[:, :, g0:g1].transpose(1, 2, 0)
        in_maps.append({
            "xs": _np_cast(np.ascontiguousarray(xs.reshape(K, XCOLS * B))),
            "wm": _np_cast(np.ascontiguousarray(wm.reshape(K, NSLOT * K))),
        })

    if DTYPE not in _prog_cache:
        _prog_cache[DTYPE] = _build_program()
    nc = _prog_cache[DTYPE]

    LAST_RESULTS = run_bass_kernel_spmd(
        nc, in_maps, list(range(N_CORES)),
        trace=bool(os.environ.get("CQT_TRACE")),
    )
    results = LAST_RESULTS.results

    # ---- host post: sum partials per block, un-permute, magnitude, dB ----
    conv_s = np.zeros((NCH, B, FRAMES), np.float64)   # sorted channel order
    for i in range(N_CORES):
        om = results[i]["om"].reshape(K, NSEG, NT, B)
        for s in range(NSEG):
            a = assign[i][s]
            if a is None:
                continue
            blk, _ = a
            ch0 = 128 * blk
            ch1 = min(ch0 + 128, NCH)
            conv_s[ch0:ch1] += om[:ch1 - ch0, s, :FRAMES, :].transpose(0, 2, 1)

    re = conv_s[0::2]                                  # [528, B, 173]
    im = conv_s[1::2]
    mag = np.sqrt(re * re + im * im)

    # ---- host refinement: exact recompute of near-silent bins ----
    conv_rms = float(np.sqrt(np.mean(mag * mag)))
    err_abs = _CONV_EPS.get(DTYPE, 1e-3) * conv_rms
    thresh = 4.343 * err_abs / DB_ERR_TARGET
    fix = np.argwhere(mag < thresh)                    # rows: (bin, b, t)
    if len(fix):
        xp64 = x_pad.astype(np.float64)
        for b in range(B):
            sel = fix[fix[:, 1] == b]
            if not len(sel):
                continue
            for t in np.unique(sel[:, 2]):
                bins = sel[sel[:, 2] == t][:, 0]
                win = xp64[b, t * HOP:t * HOP + L_in]
                re[bins, b, t] = kern_r[bins].astype(np.float64) @ win
                im[bins, b, t] = kern_i[bins].astype(np.float64) @ win
        mag = np.sqrt(re * re + im * im)

    ref = max(mag.max(), AMIN)
    log_spec = 10.0 * np.log10(np.maximum(mag, AMIN)) - 10.0 * np.log10(ref)
    log_spec = np.maximum(log_spec, log_spec.max() - TOP_DB)
    return np.ascontiguousarray(log_spec.transpose(1, 2, 0)).astype(np.float32)
